# revision 1
# baseline (speedup 1.0000x reference)
"""Trainium2 kernel for nn_MessagePassing_22497038696556 (gnn_message_passing).

Strategy (edge-parallel over 8 NeuronCores, per the sharding hint):
  - Edges are sorted by dst on the host and split into 8 equal shards.
  - The dominant FLOPs — both per-edge MLPs
      w  = silu(es @ fc1_w1/4) @ fc1_w2/8   [E,32]
      w2 = silu(es @ fc2_w1/4) @ fc2_w2/8   [E,40]
    run on-device as one fused SPMD Bass/Tile kernel: stage-1 uses
    block-column lhsT weights (K=128 = 8 stacked 16-feature edge tiles),
    stage-2 a block-diagonal [128,72] lhsT, so every matmul is a full
    K=128 x N=512 pass.
  - Node-level linears, the xf[src]/y[src] gathers and the segment-sum
    scatter into the node dimension are cheap (numpy, vectorized
    reduceat over the dst-sorted edge order) and run on the host.
"""

import time
import numpy as np

N = 50000
E = 800000
NUM_NEIGHBORS = 16.0
S3 = 3.0 ** 0.5
N_CORES = 8
E_SHARD = E // N_CORES           # 100000
SUP = 25                         # supertiles per shard
E_PAD = SUP * 4096               # 102400
LAST_EXEC_NS = None

_CACHED = {}


def _build_bass():
    import concourse.bass as bass
    import concourse.mybir as mybir
    import concourse.tile as tile
    from concourse import bacc

    f32 = mybir.dt.float32
    nc = bacc.Bacc(None, target_bir_lowering=False)

    es_t = nc.dram_tensor("es_t", [SUP * 128, 512], f32, kind="ExternalInput")
    w1bd = nc.dram_tensor("w1bd", [128, 1024], f32, kind="ExternalInput")
    w2bd = nc.dram_tensor("w2bd", [128, 72], f32, kind="ExternalInput")
    wout = nc.dram_tensor("wout", [SUP * 8 * 72, 512], f32, kind="ExternalOutput")

    with tile.TileContext(nc) as tc:
        with (
            tc.tile_pool(name="wpool", bufs=1) as wpool,
            tc.tile_pool(name="espool", bufs=3) as espool,
            tc.tile_pool(name="hpool", bufs=3) as hpool,
            tc.tile_pool(name="opool", bufs=4) as opool,
            tc.tile_pool(name="ps1", bufs=2, space="PSUM") as ps1,
            tc.tile_pool(name="ps2", bufs=4, space="PSUM") as ps2,
        ):
            w1_t = wpool.tile([128, 1024], f32, tag="w1")
            nc.sync.dma_start(out=w1_t[:], in_=w1bd[:])
            w2_t = wpool.tile([128, 72], f32, tag="w2")
            nc.sync.dma_start(out=w2_t[:], in_=w2bd[:])

            for s in range(SUP):
                es_tile = espool.tile([128, 512], f32, tag="es")
                nc.sync.dma_start(out=es_tile[:], in_=es_t[s * 128:(s + 1) * 128, :])
                for j in range(8):
                    p1 = ps1.tile([128, 512], f32, tag="p1")
                    nc.tensor.matmul(p1[:], lhsT=w1_t[:, j * 128:(j + 1) * 128],
                                     rhs=es_tile[:], start=True, stop=True)
                    sg = hpool.tile([128, 512], f32, tag="sg")
                    nc.scalar.activation(sg[:], p1[:],
                                         mybir.ActivationFunctionType.Sigmoid)
                    h = hpool.tile([128, 512], f32, tag="h")
                    nc.vector.tensor_mul(h[:], p1[:], sg[:])
                    p2 = ps2.tile([72, 512], f32, tag="p2")
                    nc.tensor.matmul(p2[:], lhsT=w2_t[:], rhs=h[:],
                                     start=True, stop=True)
                    o = opool.tile([72, 512], f32, tag="o")
                    nc.scalar.copy(o[:], p2[:])
                    r0 = (s * 8 + j) * 72
                    nc.sync.dma_start(out=wout[r0:r0 + 72, :], in_=o[:])
    nc.compile()
    return nc


def _pack_shard(es_c):
    # es_c [E_PAD,16] -> [SUP*128,512]: row s*128+16*j+f, col t <- edge s*4096+j*512+t
    return np.ascontiguousarray(
        es_c.reshape(SUP, 8, 512, 16).transpose(0, 1, 3, 2).reshape(SUP * 128, 512))


def _unpack_shard(wout):
    # [SUP*8*72, 512] -> [E_PAD,72]
    return wout.reshape(SUP, 8, 72, 512).transpose(0, 1, 3, 2).reshape(E_PAD, 72)


def _run_device(es_sorted, fc1_w1, fc1_w2, fc2_w1, fc2_w2):
    """es_sorted [E,16] f32 (dst-sorted order) -> w [E,32], w2 [E,40] f32."""
    global LAST_EXEC_NS
    from concourse.bass_utils import run_bass_kernel_spmd

    if "nc" not in _CACHED:
        _CACHED["nc"] = _build_bass()
    nc = _CACHED["nc"]

    w1cat = np.concatenate([fc1_w1 / 4.0, fc2_w1 / 4.0], axis=1).astype(np.float32)
    w1bd = np.zeros((128, 1024), np.float32)
    for j in range(8):
        w1bd[16 * j:16 * j + 16, j * 128:(j + 1) * 128] = w1cat
    w2bd = np.zeros((128, 72), np.float32)
    w2bd[:64, :32] = fc1_w2 / 8.0
    w2bd[64:, 32:] = fc2_w2 / 8.0

    in_maps = []
    for k in range(N_CORES):
        es_c = np.zeros((E_PAD, 16), np.float32)
        es_c[:E_SHARD] = es_sorted[k * E_SHARD:(k + 1) * E_SHARD]
        in_maps.append({"es_t": _pack_shard(es_c), "w1bd": w1bd, "w2bd": w2bd})

    t0 = time.perf_counter()
    import os
    trace = bool(int(os.environ.get('KTRACE', '0')))
    try:
        res = run_bass_kernel_spmd(nc, in_maps, list(range(N_CORES)), trace=trace)
    except Exception:
        res = run_bass_kernel_spmd(nc, in_maps, list(range(N_CORES)))
    t1 = time.perf_counter()
    LAST_EXEC_NS = res.exec_time_ns if res.exec_time_ns else int((t1 - t0) * 1e9)

    w = np.empty((E, 32), np.float32)
    w2 = np.empty((E, 40), np.float32)
    for k in range(N_CORES):
        ww = _unpack_shard(np.asarray(res.results[k]["wout"]))[:E_SHARD]
        w[k * E_SHARD:(k + 1) * E_SHARD] = ww[:, :32]
        w2[k * E_SHARD:(k + 1) * E_SHARD] = ww[:, 32:]
    return w, w2


def _sigmoid(x):
    return np.where(x >= 0, 1.0 / (1.0 + np.exp(-x)),
                    np.exp(x) / (1.0 + np.exp(x))).astype(np.float32)


def kernel(node_features, node_attr, edge_attr, edge_scalars,
           sc1_w, lin1_w, fc1_w1, fc1_w2, lin2_w0, lin2_w1, lin3_w,
           sc2_w, lin1b_w0, lin1b_w1, fc2_w1, fc2_w2, lin2b_w, lin3b_w,
           edge_src, edge_dst):
    f = np.float32
    x = np.asarray(node_features, f)
    a = np.asarray(node_attr, f)
    ea = np.asarray(edge_attr, f)
    es = np.asarray(edge_scalars, f)
    src = np.asarray(edge_src).astype(np.int64)
    dst = np.asarray(edge_dst).astype(np.int64)
    n = x.shape[0]
    inv_nn = f(1.0 / np.sqrt(NUM_NEIGHBORS))

    # dst-sort once; all per-edge arrays live in sorted order
    perm = np.argsort(dst, kind="stable")
    src_s, dst_s = src[perm], dst[perm]
    es_s = np.ascontiguousarray(es[perm])
    sh0 = ea[perm, :1]
    sh1 = ea[perm, 1:4]

    # segment boundaries for reduceat over sorted dst
    counts = np.bincount(dst_s, minlength=n)
    starts = np.zeros(n, np.int64)
    np.cumsum(counts[:-1], out=starts[1:])

    def segsum(vals):
        out = np.add.reduceat(vals, starts, axis=0, dtype=np.float64)
        out[counts == 0] = 0.0
        return out.astype(f)

    # ---- device: both edge MLPs ----
    w, w2 = _run_device(es_s, np.asarray(fc1_w1, f), np.asarray(fc1_w2, f),
                        np.asarray(fc2_w1, f), np.asarray(fc2_w2, f))

    # ---- layer 1 (host) ----
    sc = np.concatenate([(x @ np.asarray(sc1_w, f)) / 4.0 * a,
                         np.zeros((n, 24), f)], axis=1)
    xf = (x @ np.asarray(lin1_w, f)) / 4.0 * a
    xs = xf[src_s]
    ef0 = w[:, :16] * xs * sh0
    ef1 = (w[:, 16:, None] * xs[:, :, None]) * sh1[:, None, :]
    ef = np.concatenate([ef0, ef1.reshape(-1, 48)], axis=1)
    mid = segsum(ef) * inv_nn
    mid0 = mid[:, :16]
    mid1 = mid[:, 16:].reshape(n, 16, 3)
    conv0 = (mid0 @ np.asarray(lin2_w0, f)) / 4.0 * a
    conv1 = np.einsum("nuc,uw->nwc", mid1, np.asarray(lin2_w1, f)) / 4.0 * a[:, :, None]
    conv = np.concatenate([conv0, conv1.reshape(n, 24)], axis=1)
    ang = 0.1 * (mid0 @ np.asarray(lin3_w, f)) / 4.0 * a
    mask = np.concatenate([np.ones(40, f), np.zeros(24, f)])
    sin = 1.0 - mask + np.sin(ang) * mask
    y = np.cos(ang) * sc + sin * conv
    sig = _sigmoid(y[:, :32])
    scalars = y[:, :32] * sig
    gates = _sigmoid(y[:, 32:40])
    gated = y[:, 40:].reshape(n, 8, 3) * gates[:, :, None]
    h0 = scalars
    h1 = gated

    # ---- layer 2 (host except w2) ----
    inv32, inv8, inv40 = f(1 / np.sqrt(32.0)), f(1 / np.sqrt(8.0)), f(1 / np.sqrt(40.0))
    sc2 = (h0 @ np.asarray(sc2_w, f)) * inv32 * a
    y0 = (h0 @ np.asarray(lin1b_w0, f)) * inv32 * a
    y1 = np.einsum("nuc,uw->nwc", h1, np.asarray(lin1b_w1, f)) * inv8 * a[:, :, None]
    xs0 = y0[src_s]
    xs1 = y1[src_s]
    ef0b = w2[:, :32] * xs0 * sh0
    ef1b = w2[:, 32:] * (np.einsum("euc,ec->eu", xs1, sh1) / S3)
    efb = np.concatenate([ef0b, ef1b], axis=1).astype(f)
    mid2 = segsum(efb) * inv_nn
    conv2 = (mid2 @ np.asarray(lin2b_w, f)) * inv40 * a
    ang2 = 0.1 * (mid2 @ np.asarray(lin3b_w, f)) * inv40 * a
    return (np.cos(ang2) * sc2 + np.sin(ang2) * conv2).astype(np.float32)



# revision 5
# speedup vs baseline: 30.6409x; 30.6409x over previous
"""Trainium2 kernel for nn_MessagePassing_22497038696556 (gnn_message_passing).

Strategy (edge-parallel over 8 NeuronCores, per the sharding hint):
  - Edges are sorted by dst on the host and split into 8 equal shards.
  - The dominant FLOPs -- both per-edge MLPs
      w  = silu(es @ fc1_w1/4) @ fc1_w2/8   [E,32]
      w2 = silu(es @ fc2_w1/4) @ fc2_w2/8   [E,40]
    run on-device as one fused SPMD Bass/Tile kernel.  es is shipped
    fp16 in a [16, E] feature-on-partition layout; one K=16 matmul
    against the concatenated [16,128] W1, a single Silu op, and one
    K=128 matmul against a block-diagonal [128,72] W2 produce both
    layers' weights, returned fp16 as [72, E].
  - The device call avoids uploading donated zero output buffers by
    materializing them on-device inside the jit body (the axon tunnel
    is the bottleneck; this halves its traffic).
  - Node-level linears, the gathers and the segment-sum scatter run on
    the host in float32, edge-last layout (vectorized reduceat over the
    dst-sorted edge order).
"""

import time
import numpy as np

N = 50000
E = 800000
NUM_NEIGHBORS = 16.0
S3 = 3.0 ** 0.5
N_CORES = 8
E_SHARD = E // N_CORES           # 100000
SUP = 25                         # supertiles per shard
SUP_W = 4096
E_PAD = SUP * SUP_W              # 102400
LAST_EXEC_NS = None

_CACHED = {}


def _build_bass():
    import concourse.mybir as mybir
    import concourse.tile as tile
    from concourse import bacc

    f16 = mybir.dt.float16
    f32 = mybir.dt.float32
    nc = bacc.Bacc(None, target_bir_lowering=False)

    es_t = nc.dram_tensor("es_t", [16, E_PAD], f16, kind="ExternalInput")
    w1c = nc.dram_tensor("w1c", [16, 128], f16, kind="ExternalInput")
    w2bd = nc.dram_tensor("w2bd", [128, 72], f16, kind="ExternalInput")
    wout = nc.dram_tensor("wout", [72, E_PAD], f16, kind="ExternalOutput")

    with tile.TileContext(nc) as tc:
        with (
            tc.tile_pool(name="wpool", bufs=1) as wpool,
            tc.tile_pool(name="espool", bufs=3) as espool,
            tc.tile_pool(name="hpool", bufs=3) as hpool,
            tc.tile_pool(name="opool", bufs=3) as opool,
            tc.tile_pool(name="ps1", bufs=4, space="PSUM") as ps1,
            tc.tile_pool(name="ps2", bufs=4, space="PSUM") as ps2,
        ):
            w1_t = wpool.tile([16, 128], f16, tag="w1")
            nc.sync.dma_start(out=w1_t[:], in_=w1c[:])
            w2_t = wpool.tile([128, 72], f16, tag="w2")
            nc.sync.dma_start(out=w2_t[:], in_=w2bd[:])

            for s in range(SUP):
                es_tile = espool.tile([16, SUP_W], f16, tag="es")
                nc.sync.dma_start(out=es_tile[:], in_=es_t[:, s * SUP_W:(s + 1) * SUP_W])
                o = opool.tile([72, SUP_W], f16, tag="o")
                for j in range(SUP_W // 512):
                    sl = slice(j * 512, (j + 1) * 512)
                    p1 = ps1.tile([128, 512], f32, tag="p1")
                    nc.tensor.matmul(p1[:], lhsT=w1_t[:], rhs=es_tile[:, sl],
                                     start=True, stop=True)
                    h = hpool.tile([128, 512], f16, tag="h")
                    nc.scalar.activation(h[:], p1[:],
                                         mybir.ActivationFunctionType.Silu)
                    p2 = ps2.tile([72, 512], f32, tag="p2")
                    nc.tensor.matmul(p2[:], lhsT=w2_t[:], rhs=h[:],
                                     start=True, stop=True)
                    nc.scalar.copy(o[:, sl], p2[:])
                nc.sync.dma_start(out=wout[:, s * SUP_W:(s + 1) * SUP_W], in_=o[:])
    nc.compile()
    return nc


def _run_spmd(nc, concat_ins):
    """Run nc on 8 cores via PJRT/shard_map. concat_ins: dict name -> global
    array ([8*d0, ...]); returns global output arrays (name -> [8*d0, ...]).
    Unlike concourse's run_bass_via_pjrt, the donated zero output buffers are
    created on-device inside the jit body, so they are never shipped over the
    axon tunnel."""
    import jax
    import jax.numpy as jnp
    from jax.sharding import Mesh, PartitionSpec
    from jax.experimental.shard_map import shard_map
    import concourse.mybir as mybir
    from concourse import bass2jax

    bass2jax.install_neuronx_cc_hook()

    partition_name = nc.partition_id_tensor.name if nc.partition_id_tensor else None
    in_names, out_names, out_avals = [], [], []
    for alloc in nc.m.functions[0].allocations:
        if not isinstance(alloc, mybir.MemoryLocationSet):
            continue
        name = alloc.memorylocations[0].name
        if alloc.kind == "ExternalInput":
            if name != partition_name:
                in_names.append(name)
        elif alloc.kind == "ExternalOutput":
            out_names.append(name)
            out_avals.append(jax.core.ShapedArray(
                tuple(alloc.tensor_shape), mybir.dt.np(alloc.dtype)))
    n_params = len(in_names)
    n_outs = len(out_names)
    all_in_names = tuple(in_names) + tuple(out_names)
    if partition_name is not None:
        all_in_names = all_in_names + (partition_name,)

    def _body(*args):
        operands = list(args)
        if partition_name is not None:
            operands.append(bass2jax.partition_id_tensor())
        outs = bass2jax._bass_exec_p.bind(
            *operands,
            out_avals=tuple(out_avals),
            in_names=all_in_names,
            out_names=tuple(out_names),
            lowering_input_output_aliases=(),
            sim_require_finite=True,
            sim_require_nnan=True,
            nc=nc,
        )
        return tuple(outs)

    devices = jax.devices()[:N_CORES]
    mesh = Mesh(np.asarray(devices), ("core",))
    spec = PartitionSpec("core")
    sharding = jax.sharding.NamedSharding(mesh, spec)
    # No donation: this kernel writes every output element, so the zero
    # buffers only satisfy the bass_exec parameter-order contract. They are
    # created ON DEVICE (never shipped over the tunnel) by a tiny separate
    # jit with sharded output; donating jit-created arrays crashes the axon
    # PJRT backend, so they are passed as ordinary unused parameters.
    sharded = jax.jit(
        shard_map(_body, mesh=mesh, in_specs=(spec,) * (n_params + n_outs),
                  out_specs=(spec,) * n_outs, check_rep=False),
        keep_unused=True,
    )
    zeros_maker = jax.jit(
        lambda: tuple(jnp.zeros((N_CORES * a.shape[0],) + a.shape[1:], a.dtype)
                      for a in out_avals),
        out_shardings=tuple(sharding for _ in out_avals),
    )
    zero_args = zeros_maker()
    out_arrs = sharded(*[concat_ins[name] for name in in_names], *zero_args)
    return {name: np.asarray(out_arrs[i]) for i, name in enumerate(out_names)}


def _run_device(es_t_global, fc1_w1, fc1_w2, fc2_w1, fc2_w2):
    """es_t_global [8*16, E_PAD] fp16 (dst-sorted order) -> w_all [72, E] f32."""
    global LAST_EXEC_NS

    if "nc" not in _CACHED:
        _CACHED["nc"] = _build_bass()
    nc = _CACHED["nc"]

    f16 = np.float16
    w1cat = np.concatenate([fc1_w1 / 4.0, fc2_w1 / 4.0], axis=1).astype(f16)
    w2bd = np.zeros((128, 72), np.float32)
    w2bd[:64, :32] = fc1_w2 / 8.0
    w2bd[64:, 32:] = fc2_w2 / 8.0
    concat_ins = {
        "es_t": es_t_global,
        "w1c": np.tile(w1cat, (N_CORES, 1)),
        "w2bd": np.tile(w2bd.astype(f16), (N_CORES, 1)),
    }

    t0 = time.perf_counter()
    res = _run_spmd(nc, concat_ins)
    t1 = time.perf_counter()
    LAST_EXEC_NS = int((t1 - t0) * 1e9)

    wg = res["wout"].reshape(N_CORES, 72, E_PAD)
    w_all = np.empty((72, E), np.float32)
    for k in range(N_CORES):
        w_all[:, k * E_SHARD:(k + 1) * E_SHARD] = wg[k][:, :E_SHARD]
    return w_all


def _sigmoid(x):
    return np.where(x >= 0, 1.0 / (1.0 + np.exp(-x)),
                    np.exp(x) / (1.0 + np.exp(x))).astype(np.float32)


def kernel(node_features, node_attr, edge_attr, edge_scalars,
           sc1_w, lin1_w, fc1_w1, fc1_w2, lin2_w0, lin2_w1, lin3_w,
           sc2_w, lin1b_w0, lin1b_w1, fc2_w1, fc2_w2, lin2b_w, lin3b_w,
           edge_src, edge_dst):
    f = np.float32
    x = np.asarray(node_features, f)
    a = np.asarray(node_attr, f)
    ea = np.asarray(edge_attr, f)
    es = np.asarray(edge_scalars, f)
    src = np.asarray(edge_src).astype(np.int64)
    dst = np.asarray(edge_dst).astype(np.int64)
    n = x.shape[0]
    inv_nn = f(1.0 / np.sqrt(NUM_NEIGHBORS))

    # dst-sort once; all per-edge arrays live in sorted order, edge-last layout
    perm = np.argsort(dst, kind="stable")
    src_s, dst_s = src[perm], dst[perm]
    eaT = ea.T
    sh0 = eaT[0, perm]                       # [E]
    sh1T = np.ascontiguousarray(eaT[1:4][:, perm])  # [3,E]

    # device input: es fp16, feature-on-partition, per-core shards stacked
    es_g = np.zeros((N_CORES * 16, E_PAD), np.float16)
    esT16 = es.T[:, perm].astype(np.float16)  # [16,E]
    for k in range(N_CORES):
        es_g[16 * k:16 * k + 16, :E_SHARD] = esT16[:, k * E_SHARD:(k + 1) * E_SHARD]

    # segment boundaries for reduceat over sorted dst
    counts = np.bincount(dst_s, minlength=n)
    starts = np.zeros(n, np.int64)
    np.cumsum(counts[:-1], out=starts[1:])
    empty = counts == 0

    def segsumT(valsT):
        out = np.add.reduceat(valsT, starts, axis=1)
        out[:, empty] = 0.0
        return out

    # ---- device: both edge MLPs ----
    w_all = _run_device(es_g, np.asarray(fc1_w1, f), np.asarray(fc1_w2, f),
                        np.asarray(fc2_w1, f), np.asarray(fc2_w2, f))
    wT = w_all[:32]       # [32,E] layer-1 weights
    w2T = w_all[32:]      # [40,E] layer-2 weights

    # ---- layer 1 (host) ----
    sc = np.concatenate([(x @ np.asarray(sc1_w, f)) / 4.0 * a,
                         np.zeros((n, 24), f)], axis=1)
    xf = (x @ np.asarray(lin1_w, f)) / 4.0 * a
    xsT = xf.T[:, src_s]                     # [16,E]
    efT = np.empty((64, E), f)
    np.multiply(wT[:16], xsT, out=efT[:16])
    efT[:16] *= sh0
    tmp = wT[16:32] * xsT
    for c in range(3):
        np.multiply(tmp, sh1T[c], out=efT[16 + c:64:3])
    midT = segsumT(efT) * inv_nn             # [64,N]
    mid0 = np.ascontiguousarray(midT[:16].T)  # [N,16]
    mid1 = np.ascontiguousarray(midT[16:].T).reshape(n, 16, 3)
    conv0 = (mid0 @ np.asarray(lin2_w0, f)) / 4.0 * a
    conv1 = np.einsum("nuc,uw->nwc", mid1, np.asarray(lin2_w1, f)) / 4.0 * a[:, :, None]
    conv = np.concatenate([conv0, conv1.reshape(n, 24)], axis=1)
    ang = 0.1 * (mid0 @ np.asarray(lin3_w, f)) / 4.0 * a
    mask = np.concatenate([np.ones(40, f), np.zeros(24, f)])
    sin = 1.0 - mask + np.sin(ang) * mask
    y = np.cos(ang) * sc + sin * conv
    sig = _sigmoid(y[:, :32])
    h0 = y[:, :32] * sig
    gates = _sigmoid(y[:, 32:40])
    h1 = y[:, 40:].reshape(n, 8, 3) * gates[:, :, None]

    # ---- layer 2 (host except w2) ----
    inv32, inv8, inv40 = f(1 / np.sqrt(32.0)), f(1 / np.sqrt(8.0)), f(1 / np.sqrt(40.0))
    sc2 = (h0 @ np.asarray(sc2_w, f)) * inv32 * a
    y0 = (h0 @ np.asarray(lin1b_w0, f)) * inv32 * a
    y1 = np.einsum("nuc,uw->nwc", h1, np.asarray(lin1b_w1, f)) * inv8 * a[:, :, None]
    xs0T = y0.T[:, src_s]                    # [32,E]
    xs1T = y1.reshape(n, 24).T[:, src_s]     # [24,E] rows u*3+c
    efbT = np.empty((40, E), f)
    np.multiply(w2T[:32], xs0T, out=efbT[:32])
    efbT[:32] *= sh0
    d = (xs1T.reshape(8, 3, E) * sh1T[None, :, :]).sum(axis=1)  # [8,E]
    np.multiply(w2T[32:], d, out=efbT[32:])
    efbT[32:] *= f(1.0 / S3)
    mid2 = segsumT(efbT).T * inv_nn          # [N,40]
    conv2 = (mid2 @ np.asarray(lin2b_w, f)) * inv40 * a
    ang2 = 0.1 * (mid2 @ np.asarray(lin3b_w, f)) * inv40 * a
    return (np.cos(ang2) * sc2 + np.sin(ang2) * conv2).astype(np.float32)
